# revision 3
# baseline (speedup 1.0000x reference)
"""GAT + BN/FFN/BN kernel for TRN2, SPMD over 8 NeuronCores — v3.

Design:
  - fp16 rec table (512B rows: feat 128 | el 8 | er 8 | pad), split into
    lo/hi halves (int16 dma_gather indices). Row permutation:
    prow(n) = (n%128)*NBLKA + n//128, lo = partitions 0..63.
  - Stage A: host-transposed xT fp16; one matmul per 128-node block
    (lhsT = xT block, rhs = W_ext [W | W@AL | W@AR]); full 512B-row
    contiguous writes.
  - Edge phase per 128-dst chunk: TWO chunk-level dma_gathers (lo/hi,
    ~1us Pool SWDGE each) fetch [feat|el|er] rows for all edges;
    onehot built as one DVE compare; er[dst] broadcast to edges via
    fp16 PE transposes of the onehot + tiny matmuls (erc from resident
    x_shardT); segment softmax + message sum via onehot matmuls into
    PSUM; h = x + rst.
  - BN stats via ACT accum + cross-core AllReduce; FFN fp16; BN2.
"""
import numpy as np
from contextlib import ExitStack

import concourse.bass as bass
import concourse.tile as tile
import concourse.bacc as bacc
from concourse import mybir
from concourse.masks import make_identity

F32 = mybir.dt.float32
F16 = mybir.dt.float16
I16 = mybir.dt.int16
AF = mybir.ActivationFunctionType
OP = mybir.AluOpType

P = 128
EMBED = 128
HEADS = 8
HEAD_DIM = 16
HIDDEN = 512
REC = 144          # valid row prefix: feat 128 | el 8 | er 8
ROW = 256          # stored row elems (512B)
SLOPE = 0.2
EPS = 1e-5


def _wrap16(vals):
    """int16 index seq -> dma_gather layout [128, len/16]."""
    n = len(vals)
    base = vals.reshape(n // 16, 16).T        # [16, n/16]
    return np.tile(base, (8, 1)).astype(np.int16)


def host_prep(x, src, dst, W, attn_l, attn_r, gamma1, beta1, gamma2, beta2,
              W1, b1, W2, b2, n_cores):
    N = x.shape[0]
    assert N % n_cores == 0
    NPC = N // n_cores
    n_chunks = (NPC + P - 1) // P
    NPC_pad = n_chunks * P
    N_pad = ((N + 1023) // 1024) * 1024
    NBLKA = N_pad // P
    NHALF = (P // 2) * NBLKA

    src = np.asarray(src).astype(np.int64)
    dst = np.asarray(dst).astype(np.int64)
    x = np.asarray(x, dtype=np.float32)

    order = np.argsort(dst, kind="stable")
    srcs = src[order]
    dsts = dst[order]
    sprow = (srcs % P) * NBLKA + srcs // P    # permuted rec row of src
    slo = sprow < NHALF

    chunk_ranges = []
    max_lo = max_hi = 0
    for c in range(n_cores):
        for k in range(n_chunks):
            g0 = c * NPC + k * P
            vk = min(P, NPC - k * P)
            e0 = np.searchsorted(dsts, g0, side="left")
            e1 = np.searchsorted(dsts, g0 + vk, side="left")
            nlo = int(np.count_nonzero(slo[e0:e1]))
            max_lo = max(max_lo, nlo)
            max_hi = max(max_hi, (e1 - e0) - nlo)
            chunk_ranges.append((c, k, g0, e0, e1))
    CB_LO = max(1, (max_lo + P - 1) // P)
    CB_HI = max(1, (max_hi + P - 1) // P)
    CB = CB_LO + CB_HI
    NBLK = n_chunks * CB

    xT = np.zeros((EMBED, N_pad), np.float16)
    xT[:, :N] = x.T.astype(np.float16)

    W = np.asarray(W, np.float64)
    al = np.asarray(attn_l, np.float64)
    ar = np.asarray(attn_r, np.float64)
    ALp = np.zeros((EMBED, HEADS))
    ARp = np.zeros((EMBED, HEADS))
    for h in range(HEADS):
        ALp[h * HEAD_DIM:(h + 1) * HEAD_DIM, h] = al[h]
        ARp[h * HEAD_DIM:(h + 1) * HEAD_DIM, h] = ar[h]
    W_ext = np.concatenate([W, W @ ALp, W @ ARp], axis=1).astype(np.float16)

    in_maps = []
    for c in range(n_cores):
        idx16 = np.zeros((P, NBLK * 8), np.int16)
        dlT = np.full((NBLK * P,), 999.0, np.float16)
        for (cc, k, g0, e0, e1) in chunk_ranges:
            if cc != c:
                continue
            m = slo[e0:e1]
            plo = sprow[e0:e1][m]
            phi = sprow[e0:e1][~m] - NHALF
            dloc = (dsts[e0:e1] - g0).astype(np.float16)
            lo_seq = np.zeros((CB_LO * P,), np.int64)
            lo_seq[:len(plo)] = plo
            hi_seq = np.zeros((CB_HI * P,), np.int64)
            hi_seq[:len(phi)] = phi
            idx16[:, k * CB * 8:(k * CB + CB_LO) * 8] = \
                _wrap16(lo_seq.astype(np.int16))
            idx16[:, (k * CB + CB_LO) * 8:(k + 1) * CB * 8] = \
                _wrap16(hi_seq.astype(np.int16))
            base = k * CB * P
            dlT[base:base + len(plo)] = dloc[m]
            hbase = base + CB_LO * P
            dlT[hbase:hbase + len(phi)] = dloc[~m]
        dlT = dlT.reshape(NBLK, P).T.copy()
        x_shard = np.zeros((NPC_pad, EMBED), np.float32)
        x_shard[:NPC] = x[c * NPC:(c + 1) * NPC]
        xsT = np.zeros((EMBED, NPC_pad), np.float16)
        xsT[:, :NPC] = x[c * NPC:(c + 1) * NPC].T.astype(np.float16)
        in_maps.append({
            "xT": xT,
            "x_shard": x_shard,
            "x_shardT": xsT,
            "idx16": idx16,
            "dstlocT": dlT,
            "W_ext": W_ext,
            "W1": np.asarray(W1, np.float16),
            "W2": np.asarray(W2, np.float16),
            "b1": np.asarray(b1, np.float32).reshape(HIDDEN, 1),
            "b2": np.asarray(b2, np.float32).reshape(EMBED, 1),
            "g1": np.asarray(gamma1, np.float32).reshape(EMBED, 1),
            "be1": np.asarray(beta1, np.float32).reshape(EMBED, 1),
            "g2": np.asarray(gamma2, np.float32).reshape(EMBED, 1),
            "be2": np.asarray(beta2, np.float32).reshape(EMBED, 1),
        })
    params = dict(N=N, N_pad=N_pad, NPC=NPC, NPC_pad=NPC_pad,
                  n_chunks=n_chunks, CB=CB, CB_LO=CB_LO, NBLK=NBLK,
                  n_cores=n_cores)
    return params, in_maps


def build(params, mode='full', reps=1):
    N = params["N"]
    N_pad = params["N_pad"]
    NPC = params["NPC"]
    NPC_pad = params["NPC_pad"]
    n_chunks = params["n_chunks"]
    CB = params["CB"]
    CB_LO = params["CB_LO"]
    CB_HI = CB - CB_LO
    NBLK = params["NBLK"]
    n_cores = params["n_cores"]
    NBLKA = N_pad // P
    NHALF = (P // 2) * NBLKA

    nc = bacc.Bacc("TRN2", target_bir_lowering=False, debug=False,
                   num_devices=n_cores)

    dt = lambda name, shape, dtype=F32, kind="ExternalInput": \
        nc.dram_tensor(name, shape, dtype, kind=kind).ap()

    xT_in = dt("xT", [EMBED, N_pad], F16)
    x_shard = dt("x_shard", [NPC_pad, EMBED])
    xsT_in = dt("x_shardT", [EMBED, NPC_pad], F16)
    idx16_in = dt("idx16", [P, NBLK * 8], I16)
    dstlocT = dt("dstlocT", [P, NBLK], F16)
    Wext_in = dt("W_ext", [EMBED, REC], F16)
    W1_in = dt("W1", [EMBED, HIDDEN], F16)
    W2_in = dt("W2", [HIDDEN, EMBED], F16)
    b1_in = dt("b1", [HIDDEN, 1])
    b2_in = dt("b2", [EMBED, 1])
    g1_in = dt("g1", [EMBED, 1])
    be1_in = dt("be1", [EMBED, 1])
    g2_in = dt("g2", [EMBED, 1])
    be2_in = dt("be2", [EMBED, 1])
    out_shard = dt("out", [NPC_pad, EMBED], kind="ExternalOutput")

    rec_lo = nc.dram_tensor("rec_lo", [NHALF, ROW], F16, kind="Internal").ap()
    rec_hi = nc.dram_tensor("rec_hi", [NHALF, ROW], F16, kind="Internal").ap()
    rec_lo3 = rec_lo.rearrange("(p g) f -> p g f", p=P // 2)
    rec_hi3 = rec_hi.rearrange("(p g) f -> p g f", p=P // 2)

    with tile.TileContext(nc) as tc, ExitStack() as ctx:
        const = ctx.enter_context(tc.tile_pool(name="const", bufs=1))
        sbA = ctx.enter_context(tc.tile_pool(name="sbA", bufs=3))
        gQ = ctx.enter_context(tc.tile_pool(name="gQ", bufs=3))
        ohp = ctx.enter_context(tc.tile_pool(name="ohp", bufs=2))
        wmp = ctx.enter_context(tc.tile_pool(name="wmp", bufs=2))
        sbC = ctx.enter_context(tc.tile_pool(name="sbC", bufs=3))
        ps = ctx.enter_context(tc.tile_pool(name="ps", bufs=2, space="PSUM"))
        dramp = ctx.enter_context(tc.tile_pool(name="dramp", bufs=1, space="DRAM"))

        # ---------- constants ----------
        ident = const.tile([P, P], F32)
        make_identity(nc, ident[:])
        ident16 = const.tile([P, P], F16)
        make_identity(nc, ident16[:])
        iota_rep = const.tile([P, CB, P], F16)
        nc.gpsimd.iota(iota_rep[:], pattern=[[0, CB], [1, P]], base=0,
                       channel_multiplier=0,
                       allow_small_or_imprecise_dtypes=True)
        dlT_sb = const.tile([P, NBLK], F16)
        nc.sync.dma_start(dlT_sb[:], dstlocT[:])
        xsT_sb = const.tile([P, NPC_pad], F16)
        nc.sync.dma_start(xsT_sb[:], xsT_in[:])

        Wext_sb = const.tile([P, REC], F16)
        nc.sync.dma_start(Wext_sb[:], Wext_in[:])
        W1_sb = const.tile([P, HIDDEN], F16)
        nc.sync.dma_start(W1_sb[:], W1_in[:])
        W2_sb = [const.tile([P, EMBED], F16, tag=f"w2_{i}", name=f"w2_{i}")
                 for i in range(4)]
        for i in range(4):
            nc.sync.dma_start(W2_sb[i][:], W2_in[i * P:(i + 1) * P, :])
        b1_sb = const.tile([P, 4], F32)
        nc.sync.dma_start(b1_sb[:], b1_in[:].rearrange("(a p) b -> p (a b)", p=P))
        b2_sb = const.tile([P, 1], F32)
        nc.sync.dma_start(b2_sb[:], b2_in[:])
        bn_sb = const.tile([P, 4], F32)  # g1 be1 g2 be2
        nc.sync.dma_start(bn_sb[:, 0:1], g1_in[:])
        nc.sync.dma_start(bn_sb[:, 1:2], be1_in[:])
        nc.sync.dma_start(bn_sb[:, 2:3], g2_in[:])
        nc.sync.dma_start(bn_sb[:, 3:4], be2_in[:])

        hT = const.tile([P, NPC_pad], F16)
        if NPC_pad > NPC:
            nc.vector.memset(hT[:, NPC:NPC_pad], 0.0)
        sums1 = const.tile([P, n_chunks], F32)
        sqs1 = const.tile([P, n_chunks], F32)

        for _rep in range(reps):
          # ---------- stage A ----------
          SAB = 8
          assert NBLKA % SAB == 0
          for g in range(NBLKA // SAB if mode != 'empty' else 0):
              xb = sbA.tile([P, SAB * P], F16, tag="xa")
              nc.sync.dma_start(xb[:], xT_in[:, g * SAB * P:(g + 1) * SAB * P])
              rec_sb = sbA.tile([P, SAB, ROW], F16, tag="reco")
              for j in range(SAB):
                  rec_ps = ps.tile([P, REC], F32, tag="tA")
                  nc.tensor.matmul(rec_ps[:], xb[:, j * P:(j + 1) * P],
                                   Wext_sb[:], start=True, stop=True)
                  if j % 2 == 0:
                      nc.scalar.copy(rec_sb[:, j, 0:REC], rec_ps[:])
                  else:
                      nc.vector.tensor_copy(rec_sb[:, j, 0:REC], rec_ps[:])
              cols = slice(g * SAB, (g + 1) * SAB)
              nc.scalar.dma_start(rec_lo3[:, cols, :], rec_sb[0:P // 2, :, :])
              nc.scalar.dma_start(rec_hi3[:, cols, :], rec_sb[P // 2:P, :, :])

          # ---------- edge phase ----------
          for k in range(n_chunks if mode in ('full', 'gather', 'noccl') else 0):
              vk = min(P, NPC - k * P)
              idx_sb = gQ.tile([P, CB * 8], I16, tag="idx")
              nc.sync.dma_start(idx_sb[:],
                                idx16_in[:, k * CB * 8:(k + 1) * CB * 8])
              Q = gQ.tile([P, CB, ROW], F16, tag="Q")
              # split each half's gather into <=8-block (1024-idx) pieces
              # (single dma_gather with num_idxs > 1024 wedges the device)
              for (hb0, hb1, tbl) in ((0, CB_LO, rec_lo),
                                      (CB_LO, CB, rec_hi)):
                  b0 = hb0
                  while b0 < hb1:
                      b1 = min(b0 + 8, hb1)
                      nb = b1 - b0
                      nc.gpsimd.dma_gather(
                          Q[:, b0:b1, :], tbl[:],
                          idx_sb[:, b0 * 8:b1 * 8],
                          nb * P, nb * P, ROW)
                      b0 = b1

              cols = slice(k * CB, (k + 1) * CB)
              oh = ohp.tile([P, CB, P], F16, tag="oh")
              nc.vector.tensor_tensor(
                  oh[:, :, :], iota_rep[:, :, :],
                  dlT_sb[:, cols, None].to_broadcast([P, CB, P]),
                  op=OP.is_equal)
              if mode == 'gather':
                  continue

              # erc = er of local chunk nodes  [128, 8]
              erc_ps = ps.tile([P, HEADS], F32, tag="tB")
              nc.tensor.matmul(erc_ps[:], xsT_sb[:, k * P:(k + 1) * P],
                               Wext_sb[:, EMBED + HEADS:REC],
                               start=True, stop=True)
              erc_sb = sbC.tile([P, HEADS], F16, tag="erc")
              nc.scalar.copy(erc_sb[:], erc_ps[:])

              # ohT blocks (fp16 transposes, 6 per PSUM group) -> ere
              oht_sb = ohp.tile([P, CB, P], F16, tag="oht")
              ngrp = (CB + 5) // 6
              for gidx in range(ngrp):
                  b0 = gidx * 6
                  b1 = min(b0 + 6, CB)
                  oht_ps = ps.tile([P, 6, P], F16, tag="oht")
                  for b in range(b0, b1):
                      nc.tensor.transpose(oht_ps[:, b - b0, :], oh[:, b, :],
                                          ident16[:])
                  if gidx == 0:
                      nc.scalar.copy(
                          oht_sb[:, b0:b1, :].rearrange("p b f -> p (b f)"),
                          oht_ps[:, 0:b1 - b0, :].rearrange("p b f -> p (b f)"))
                  else:
                      nc.vector.tensor_copy(
                          oht_sb[:, b0:b1, :].rearrange("p b f -> p (b f)"),
                          oht_ps[:, 0:b1 - b0, :].rearrange("p b f -> p (b f)"))
              ere_ps = ps.tile([P, CB * HEADS], F32, tag="tD")
              for b in range(CB):
                  nc.tensor.matmul(ere_ps[:, b * HEADS:(b + 1) * HEADS],
                                   oht_sb[:, b, :], erc_sb[:],
                                   start=True, stop=True)

              # e = lrelu(el + er) fused; ex = exp(e) written straight into wm
              ew = wmp.tile([P, CB, HEADS], F16, tag="ew")
              nc.vector.tensor_tensor(
                  ew[:, :, :], Q[:, :, EMBED:EMBED + HEADS],
                  ere_ps[:].rearrange("p (b h) -> p b h", h=HEADS),
                  op=OP.add)
              es = wmp.tile([P, CB, HEADS], F16, tag="es")
              nc.vector.scalar_tensor_tensor(
                  es[:, :, :], ew[:, :, :], SLOPE, ew[:, :, :],
                  op0=OP.mult, op1=OP.max)

              # wm = [feat*ex | ex]
              wm = wmp.tile([P, CB, EMBED + HEADS], F16, tag="wm")
              nc.scalar.activation(wm[:, :, EMBED:EMBED + HEADS], es[:, :, :],
                                   AF.Exp)
              nc.vector.tensor_tensor(
                  wm[:, :, 0:EMBED].rearrange("p b (h d) -> p b h d", h=HEADS),
                  Q[:, :, 0:EMBED].rearrange("p b (h d) -> p b h d", h=HEADS),
                  wm[:, :, EMBED:EMBED + HEADS, None].to_broadcast(
                      [P, CB, HEADS, HEAD_DIM]),
                  op=OP.mult)

              seg_ps = ps.tile([P, EMBED + HEADS], F32, tag="tD")
              for b in range(CB):
                  nc.tensor.matmul(seg_ps[:], oh[:, b, :], wm[:, b, :],
                                   start=(b == 0), stop=(b == CB - 1))

              xc = sbC.tile([P, EMBED], F32, tag="xc")
              nc.sync.dma_start(xc[:], x_shard[k * P:(k + 1) * P, :])
              den = sbC.tile([P, HEADS], F32, tag="den")
              nc.vector.tensor_scalar(den[:], seg_ps[:, EMBED:EMBED + HEADS],
                                      1e-30, None, op0=OP.add)
              rec_ip = sbC.tile([P, HEADS], F32, tag="recip")
              nc.vector.reciprocal(rec_ip[:], den[:])
              hsb = sbC.tile([P, EMBED], F32, tag="hsb")
              nc.vector.tensor_tensor(
                  hsb[:].rearrange("p (h d) -> p h d", h=HEADS),
                  seg_ps[:, 0:EMBED].rearrange("p (h d) -> p h d", h=HEADS),
                  rec_ip[:, :, None].to_broadcast([P, HEADS, HEAD_DIM]),
                  op=OP.mult)
              nc.vector.tensor_add(hsb[:], hsb[:], xc[:])

              ht_ps = ps.tile([P, P], F32, tag="tB")
              nc.tensor.transpose(ht_ps[:], hsb[:], ident[:])
              nc.scalar.activation(hT[:, k * P:k * P + vk], ht_ps[:, :vk],
                                   AF.Copy, accum_out=sums1[:, k:k + 1])
              scr = sbC.tile([P, P], F32, tag="scr")
              nc.scalar.activation(scr[:, :vk], ht_ps[:, :vk], AF.Square,
                                   accum_out=sqs1[:, k:k + 1])

          if mode in ('empty', 'stageA', 'gather'):
              nc.vector.memset(hT[:], 0.0)
              for k in range(n_chunks):
                  nc.vector.memset(sums1[:, k:k + 1], 0.0)
                  nc.vector.memset(sqs1[:, k:k + 1], 0.0)

          # ---------- BN1 stats ----------
          stat_loc = const.tile([P, 2], F32)
          nc.vector.reduce_sum(stat_loc[:, 0:1], sums1[:], axis=mybir.AxisListType.X)
          nc.vector.reduce_sum(stat_loc[:, 1:2], sqs1[:], axis=mybir.AxisListType.X)
          stat_g = const.tile([P, 2], F32)
          if mode == 'noccl':
              nc.vector.tensor_scalar(stat_g[:], stat_loc[:], float(n_cores),
                                      None, op0=OP.mult)
          else:
              cc_in1 = dramp.tile([P, 2], F32)
              cc_out1 = dramp.tile([P, 2], F32)
              nc.sync.dma_start(cc_in1[:], stat_loc[:])
              nc.gpsimd.collective_compute(
                  "AllReduce", OP.add, replica_groups=[list(range(n_cores))],
                  ins=[cc_in1.opt()], outs=[cc_out1.opt()])
              nc.sync.dma_start(stat_g[:], cc_out1[:])

          def bn_coeffs(stat_tile, g_col, be_col, tagpfx):
              mu = const.tile([P, 2], F32, tag=f"{tagpfx}_mu")
              nc.vector.tensor_scalar(mu[:], stat_tile[:], 1.0 / N, None, op0=OP.mult)
              musq = const.tile([P, 1], F32, tag=f"{tagpfx}_musq")
              nc.vector.tensor_tensor(musq[:], mu[:, 0:1], mu[:, 0:1], op=OP.mult)
              var = const.tile([P, 1], F32, tag=f"{tagpfx}_var")
              nc.vector.tensor_tensor(var[:], mu[:, 1:2], musq[:], op=OP.subtract)
              nc.vector.tensor_scalar(var[:], var[:], EPS, None, op0=OP.add)
              std = const.tile([P, 1], F32, tag=f"{tagpfx}_std")
              nc.scalar.activation(std[:], var[:], AF.Sqrt)
              rstd = const.tile([P, 1], F32, tag=f"{tagpfx}_rstd")
              nc.vector.reciprocal(rstd[:], std[:])
              scale = const.tile([P, 1], F32, tag=f"{tagpfx}_scale")
              nc.vector.tensor_tensor(scale[:], g_col, rstd[:], op=OP.mult)
              shift = const.tile([P, 1], F32, tag=f"{tagpfx}_shift")
              nc.vector.tensor_tensor(shift[:], mu[:, 0:1], scale[:], op=OP.mult)
              nc.vector.tensor_tensor(shift[:], be_col, shift[:], op=OP.subtract)
              return scale, shift

          sc1, sh1 = bn_coeffs(stat_g, bn_sb[:, 0:1], bn_sb[:, 1:2], "bn1")

          # ---------- FFN ----------
          tiles = []
          off = 0
          while off < NPC_pad:
              w = min(512, NPC_pad - off)
              tiles.append((off, w))
              off += w
          sums2 = const.tile([P, len(tiles)], F32)
          sqs2 = const.tile([P, len(tiles)], F32)

          for ti, (off, w) in enumerate(tiles):
              nc.scalar.activation(hT[:, off:off + w], hT[:, off:off + w],
                                   AF.Identity, bias=sh1[:], scale=sc1[:])
              yps = ps.tile([P, 512], F32, tag="tD")
              for i in range(4):
                  zps = ps.tile([P, 512], F32, tag="tA")
                  nc.tensor.matmul(zps[:, :w], W1_sb[:, i * P:(i + 1) * P],
                                   hT[:, off:off + w], start=True, stop=True)
                  zr = sbC.tile([P, 512], F16, tag="zr")
                  nc.scalar.activation(zr[:, :w], zps[:, :w], AF.Relu,
                                       bias=b1_sb[:, i:i + 1])
                  nc.tensor.matmul(yps[:, :w], W2_sb[i][:], zr[:, :w],
                                   start=(i == 0), stop=(i == 3))
              nc.vector.tensor_add(hT[:, off:off + w], hT[:, off:off + w],
                                   yps[:, :w])
              nc.vector.tensor_scalar(hT[:, off:off + w], hT[:, off:off + w],
                                      b2_sb[:, 0:1], None, op0=OP.add)
              v0 = min(off, NPC)
              v1 = min(off + w, NPC)
              if v1 > v0:
                  scr2 = sbC.tile([P, 512], F32, tag="scr2")
                  nc.scalar.activation(scr2[:, :v1 - v0], hT[:, v0:v1], AF.Copy,
                                       accum_out=sums2[:, ti:ti + 1])
                  scr3 = sbC.tile([P, 512], F32, tag="scr3")
                  nc.scalar.activation(scr3[:, :v1 - v0], hT[:, v0:v1], AF.Square,
                                       accum_out=sqs2[:, ti:ti + 1])
              else:
                  nc.vector.memset(sums2[:, ti:ti + 1], 0.0)
                  nc.vector.memset(sqs2[:, ti:ti + 1], 0.0)

          stat_loc2 = const.tile([P, 2], F32, tag="sl2")
          nc.vector.reduce_sum(stat_loc2[:, 0:1], sums2[:], axis=mybir.AxisListType.X)
          nc.vector.reduce_sum(stat_loc2[:, 1:2], sqs2[:], axis=mybir.AxisListType.X)
          stat_g2 = const.tile([P, 2], F32, tag="sg2")
          if mode == 'noccl':
              nc.vector.tensor_scalar(stat_g2[:], stat_loc2[:], float(n_cores),
                                      None, op0=OP.mult)
          else:
              cc_in2 = dramp.tile([P, 2], F32)
              cc_out2 = dramp.tile([P, 2], F32)
              nc.sync.dma_start(cc_in2[:], stat_loc2[:])
              nc.gpsimd.collective_compute(
                  "AllReduce", OP.add, replica_groups=[list(range(n_cores))],
                  ins=[cc_in2.opt()], outs=[cc_out2.opt()])
              nc.sync.dma_start(stat_g2[:], cc_out2[:])
          sc2, sh2 = bn_coeffs(stat_g2, bn_sb[:, 2:3], bn_sb[:, 3:4], "bn2")

          # ---------- BN2 + output ----------
          for k in range(n_chunks):
              ob = sbC.tile([P, P], F32, tag="ob")
              nc.scalar.activation(ob[:], hT[:, k * P:(k + 1) * P], AF.Identity,
                                   bias=sh2[:], scale=sc2[:])
              ot_ps = ps.tile([P, P], F32, tag="tB")
              nc.tensor.transpose(ot_ps[:], ob[:], ident[:])
              osb = sbC.tile([P, P], F32, tag="osb")
              nc.vector.tensor_copy(osb[:], ot_ps[:])
              nc.sync.dma_start(out_shard[k * P:(k + 1) * P, :], osb[:])

    nc.compile()
    return nc


_CACHE = {}


def _get_compiled(params):
    key = tuple(sorted((k, int(v)) for k, v in params.items()))
    if key not in _CACHE:
        _CACHE[key] = build(params)
    return _CACHE[key]


def kernel(**inputs):
    """Full-input GAT+BN/FFN/BN layer on 8 TRN2 NeuronCores (v3)."""
    from concourse import bass_utils

    n_cores = 8
    x = np.asarray(inputs["x"], np.float32)
    params, in_maps = host_prep(
        x, inputs["src"], inputs["dst"], inputs["W"],
        inputs["attn_l"], inputs["attn_r"],
        inputs["gamma1"], inputs["beta1"],
        inputs["gamma2"], inputs["beta2"],
        inputs["W1"], inputs["b1"], inputs["W2"], inputs["b2"], n_cores)
    nc = _get_compiled(params)
    res = bass_utils.run_bass_kernel_spmd(nc, in_maps,
                                          core_ids=list(range(n_cores)))
    NPC = params["NPC"]
    out = np.concatenate(
        [res.results[c]["out"][:NPC] for c in range(n_cores)], axis=0)
    return out.astype(np.float32)
